# revision 13
# baseline (speedup 1.0000x reference)
"""Trainium2 Bass kernel for nn_Attention (Bahdanau-style additive attention).

Reference computation:
    enc = encoder_outputs.transpose(1, 0, 2)            # [B, S, 2H]
    e_proj = enc @ w_e.T                                # [B, S, H]
    energy = tanh(h_proj[:, None, :] + e_proj + b)      # [B, S, H]
    att = energy @ v_w                                  # [B, S]
    out = softmax(att, axis=1)

Sharding: data-parallel over batch, 4 batch rows per core on 8 cores.

The dominant cost is the e_proj matmul (34.4 GFLOP/core). This version
runs it in fp8 (e4m3) with DoubleRow perf mode (2 fp8 MACs per PE cell
per cycle, K=256 per accumulation chunk), ~2x the bf16 matmul rate:
  - host pre-transposes + quantizes enc to fp8 [b, ec, p, i, s] tiles
    (e = ec*256 + i*128 + p), so the kernel does plain contiguous DMAs
  - w_e is quantized to fp8 [ec, p, i, h] and kept SBUF-resident
  - per s-tile: psum[s(128), h(512)] accumulates 8 DoubleRow matmuls
    (lhsT = enc chunk [128, 2, 128] stationary, rhs = w chunk
    [128, 2, 512] moving), two h-groups = two psum banks
  - epilogue off the PE: DVE adds the (scaled) bias c_b = h_proj +
    attn_b in fp32, ACT applies tanh with a 2^-13 descale folded into
    its input scale, and a single fused DVE tensor_tensor_reduce does
    the v-weighted reduction straight into the logit column
  - fp8 values are scaled into range (enc x16, w x512, both exact
    powers of two); the product scale 2^13 is removed by the ACT scale

fp8 quantization alone would put the softmax rel-err at ~1.9e-2 --
too close to the 2e-2 gate. The host therefore subtracts the COHERENT
part of the logit error, which is exactly computable with matvecs:
  delta_att(b,s) ~= sum_h v_h * tanh'(u) * delta_u(b,s,h)
                 ~= sum_h (v_h * E[tanh'|b,h]) * delta_u
  with sum_h vt_h * delta_u = enc8_row . (w8^T vt) - enc_row . (w^T vt)
E[tanh'|b,h] is a 1D Gaussian integral (Gauss-Hermite) since
u(b,s,h) ~ N(c_b[b,h], ||w_e[h]||^2) over s. This cuts the measured
rel-err to ~5.6e-3 in simulation. h_proj and the final softmax are
tiny and run on the host in fp32.
"""

import sys

try:
    import concourse.bass as bass  # noqa: F401
except ImportError:
    sys.path.insert(0, "/opt/trn_rl_repo")

import numpy as np
import ml_dtypes

import concourse.bacc as bacc
import concourse.mybir as mybir
import concourse.tile as tile
from concourse.bass_utils import run_bass_kernel_spmd

HID = 1024
BATCH = 32
SRC_LEN = 2048

N_CORES = 8
B_LOC = BATCH // N_CORES      # 4
E = 2 * HID                   # 2048
N_EC = E // 256               # 8 e-chunks of 256 (DoubleRow K)
N_ST = SRC_LEN // 128         # 16 s-tiles per batch row
HG = 512                      # h per psum bank
N_HG = HID // HG              # 2 h-groups
N_RAMP = 4                    # s-tiles computed e-chunk-major at startup

ENC_SCALE = 16.0              # exact powers of two
W_SCALE = 512.0
INV_SC = 1.0 / (ENC_SCALE * W_SCALE)

f32 = mybir.dt.float32
bf16 = mybir.dt.bfloat16
f8 = mybir.dt.float8e4
DR = mybir.MatmulPerfMode.DoubleRow

_NC_CACHE = {}


def _build():
    nc = bacc.Bacc(
        "TRN2", target_bir_lowering=False, debug=False, num_devices=N_CORES
    )
    enc8 = nc.declare_dram_parameter(
        "enc8", [B_LOC, N_EC, 128, 2, SRC_LEN], f8, isOutput=False
    )
    w8 = nc.declare_dram_parameter("w8", [N_EC, 128, 2, HID], f8, isOutput=False)
    cbb = nc.declare_dram_parameter("cbb", [B_LOC, 128, HID], f32, isOutput=False)
    vb = nc.declare_dram_parameter("vb", [128, HID], bf16, isOutput=False)
    # [b, p, st]: logit(b, st*128 + p)
    att = nc.declare_dram_parameter("att", [B_LOC, 128, N_ST], f32, isOutput=True)

    with tile.TileContext(nc) as tc:
        with (
            tc.tile_pool(name="const", bufs=1) as const_pool,
            tc.tile_pool(name="cbbp", bufs=2) as cbb_pool,
            tc.tile_pool(name="encp", bufs=16) as enc_pool,
            tc.tile_pool(name="tanhE", bufs=4) as te_pool,
            tc.tile_pool(name="prep", bufs=4) as pre_pool,
            tc.tile_pool(name="scr", bufs=3) as sc_pool,
            tc.tile_pool(name="attsb", bufs=1) as att_pool,
            tc.tile_pool(name="psum", bufs=8, space="PSUM") as psum_pool,
        ):
            w_sb = const_pool.tile([128, N_EC, 2, HID], f8)
            vb_sb = const_pool.tile([128, HID], bf16)
            att_sb = att_pool.tile([128, B_LOC * N_ST], f32)

            enc_ts = {}
            cbb_sbs = [None] * B_LOC

            def load_w(ec, split=False):
                if split:
                    for hg in range(N_HG):
                        nc.sync.dma_start(
                            w_sb[:, ec, :, hg * HG:(hg + 1) * HG],
                            w8[ec, :, :, hg * HG:(hg + 1) * HG],
                        )
                else:
                    nc.sync.dma_start(w_sb[:, ec], w8[ec])

            def load_enc(b, ec):
                t = enc_pool.tile(
                    [128, 2, SRC_LEN], f8, tag="enc", name=f"enc_{b}_{ec}"
                )
                nc.sync.dma_start(t[:], enc8[b, ec])
                enc_ts[(b, ec)] = t

            QS = SRC_LEN // 4     # ramp DMA quarter (= s-tiles 4q..4q+3)

            def load_enc_q(b, ec, q):
                if q == 0:
                    t = enc_pool.tile(
                        [128, 2, SRC_LEN], f8, tag="enc", name=f"enc_{b}_{ec}"
                    )
                    enc_ts[(b, ec)] = t
                t = enc_ts[(b, ec)]
                nc.sync.dma_start(
                    t[:, :, q * QS:(q + 1) * QS],
                    enc8[b, ec, :, :, q * QS:(q + 1) * QS],
                )

            def load_cbb(b):
                t = cbb_pool.tile([128, HID], f32, tag="cbb", name=f"cbb_{b}")
                nc.sync.dma_start(t[:], cbb[b])
                cbb_sbs[b] = t

            # startup DMAs in small e-chunk waves (w[ec] split by h-group +
            # the first enc s-quarter, ~384KB/wave ~= the PE's per-chunk
            # ramp consumption) so the first matmul starts after one wave
            # and never starves through the ramp
            for ec in range(N_EC):
                load_w(ec, split=True)
                load_enc_q(0, ec, 0)
            load_cbb(0)
            nc.sync.dma_start(vb_sb[:], vb[:])
            for q in range(1, 4):
                for ec in range(N_EC):
                    load_enc_q(0, ec, q)

            # warmup tanh for the ACT LUT-table dependency
            warm = const_pool.tile([128, 1], f32)
            nc.scalar.activation(
                warm[:], vb_sb[:, 0:1], mybir.ActivationFunctionType.Tanh
            )

            def mm(ps_hg, b, st, ec):
                lhsT = enc_ts[(b, ec)][:, :, st * 128:(st + 1) * 128]
                for hg in range(N_HG):
                    nc.tensor.matmul(
                        ps_hg[hg][:],
                        lhsT=lhsT,
                        rhs=w_sb[:, ec, :, hg * HG:(hg + 1) * HG],
                        start=(ec == 0),
                        stop=(ec == N_EC - 1),
                        perf_mode=DR,
                    )

            def epilogue(b, st, ps_hg):
                te = te_pool.tile([128, HID], bf16, tag="te", name=f"te_{b}_{st}")
                for hg in range(N_HG):
                    pre = pre_pool.tile(
                        [128, HG], f32, tag="pre", name=f"pre_{b}_{st}_{hg}"
                    )
                    nc.vector.tensor_add(
                        out=pre[:],
                        in0=ps_hg[hg][:],
                        in1=cbb_sbs[b][:, hg * HG:(hg + 1) * HG],
                    )
                    nc.scalar.activation(
                        te[:, hg * HG:(hg + 1) * HG], pre[:],
                        mybir.ActivationFunctionType.Tanh,
                        scale=INV_SC,
                    )
                col = b * N_ST + st
                prod = sc_pool.tile(
                    [128, HID], bf16, tag="ttr", name=f"ttr_{b}_{st}"
                )
                nc.vector.tensor_mul(out=prod[:], in0=te[:], in1=vb_sb[:])
                nc.vector.tensor_reduce(
                    att_sb[:, col:col + 1],
                    prod[:],
                    mybir.AxisListType.X,
                    mybir.AluOpType.add,
                )

            def epilogue_chunked(b, st, ps_hg):
                # final s-tile: its epilogue latency is fully exposed at the
                # end of the run, so pipeline add/tanh in quarter chunks and
                # reduce in halves
                te = te_pool.tile([128, HID], bf16, tag="te", name=f"te_{b}_{st}")
                col = b * N_ST + st
                tmp2 = sc_pool.tile([128, 2], f32, tag="t2", name=f"t2_{b}_{st}")
                CH = HG // 2
                for hg in range(N_HG):
                    for q in range(2):
                        lo = hg * HG + q * CH
                        pre = pre_pool.tile(
                            [128, CH], f32, tag="pre", name=f"pre_{b}_{st}_{hg}_{q}"
                        )
                        nc.vector.tensor_add(
                            out=pre[:],
                            in0=ps_hg[hg][:, q * CH:(q + 1) * CH],
                            in1=cbb_sbs[b][:, lo:lo + CH],
                        )
                        nc.scalar.activation(
                            te[:, lo:lo + CH], pre[:],
                            mybir.ActivationFunctionType.Tanh,
                            scale=INV_SC,
                        )
                    prod = sc_pool.tile(
                        [128, HG], bf16, tag="ttr", name=f"ttr_{b}_{st}_{hg}"
                    )
                    nc.vector.tensor_mul(
                        out=prod[:],
                        in0=te[:, hg * HG:(hg + 1) * HG],
                        in1=vb_sb[:, hg * HG:(hg + 1) * HG],
                    )
                    nc.vector.tensor_reduce(
                        tmp2[:, hg:hg + 1],
                        prod[:],
                        mybir.AxisListType.X,
                        mybir.AluOpType.add,
                    )
                nc.vector.tensor_reduce(
                    att_sb[:, col:col + 1],
                    tmp2[:],
                    mybir.AxisListType.X,
                    mybir.AluOpType.add,
                )

            def psum_pair(b, st):
                return [
                    psum_pool.tile([128, HG], f32, tag="ps", name=f"ps_{b}_{st}_{g}")
                    for g in range(N_HG)
                ]

            # ---- batch row 0 ramp: first N_RAMP s-tiles e-chunk-major so
            # the PE starts as soon as (w8[0], enc[0,0]) land instead of
            # waiting for the whole row
            ramp_ps = [psum_pair(0, st) for st in range(N_RAMP)]
            for ec in range(N_EC):
                for st in range(N_RAMP):
                    mm(ramp_ps[st], 0, st, ec)
            for st in range(N_RAMP):
                epilogue(0, st, ramp_ps[st])

            # ---- steady state ----
            for b in range(B_LOC):
                for st in range(0 if b else N_RAMP, N_ST):
                    # prefetch next row's tiles mid-row
                    if b < B_LOC - 1:
                        if st == 4:
                            load_cbb(b + 1)
                        if 4 <= st < 4 + N_EC:
                            load_enc(b + 1, st - 4)
                    ps_hg = psum_pair(b, st)
                    for ec in range(N_EC):
                        mm(ps_hg, b, st, ec)
                    if b == B_LOC - 1 and st == N_ST - 1:
                        epilogue_chunked(b, st, ps_hg)
                    else:
                        epilogue(b, st, ps_hg)
                nc.sync.dma_start(att[b], att_sb[:, b * N_ST:(b + 1) * N_ST])
    nc.compile()
    return nc


def _get_nc():
    if "nc" not in _NC_CACHE:
        _NC_CACHE["nc"] = _build()
    return _NC_CACHE["nc"]


def kernel(hidden, encoder_outputs, attn_w, attn_b, v_w, _trace=False):
    hidden = np.asarray(hidden, dtype=np.float32)
    encoder_outputs = np.asarray(encoder_outputs, dtype=np.float32)
    attn_w = np.asarray(attn_w, dtype=np.float32)
    attn_b = np.asarray(attn_b, dtype=np.float32)
    v_w = np.asarray(v_w, dtype=np.float32)

    c_b = hidden @ attn_w[:, :HID].T + attn_b          # [B, H] fp32
    w_e = np.ascontiguousarray(attn_w[:, HID:])        # [H, E]

    # fp8 quantization (scales are exact powers of two)
    w8_q = (w_e * np.float32(W_SCALE)).astype(ml_dtypes.float8_e4m3)   # [H, E]
    e8_q = (encoder_outputs * np.float32(ENC_SCALE)).astype(
        ml_dtypes.float8_e4m3
    )                                                                   # [S, B, E]

    # device weight layout [ec, p, i, h], e = ec*256 + i*128 + p
    w8_dev = np.ascontiguousarray(
        w8_q.T.reshape(N_EC, 2, 128, HID).transpose(0, 2, 1, 3)
    )
    vb_dev = np.ascontiguousarray(
        np.broadcast_to(v_w[None, :], (128, HID))
    ).astype(ml_dtypes.bfloat16)

    nc = _get_nc()
    sc = np.float32(ENC_SCALE * W_SCALE)
    in_maps = []
    for core in range(N_CORES):
        b0 = core * B_LOC
        enc_dev = np.ascontiguousarray(
            e8_q[:, b0:b0 + B_LOC, :].transpose(1, 2, 0)
            .reshape(B_LOC, N_EC, 2, 128, SRC_LEN).transpose(0, 1, 3, 2, 4)
        )
        cbb_dev = np.ascontiguousarray(
            np.broadcast_to(
                (c_b[b0:b0 + B_LOC] * sc)[:, None, :], (B_LOC, 128, HID)
            )
        ).astype(np.float32)
        in_maps.append(
            {"enc8": enc_dev, "w8": w8_dev, "cbb": cbb_dev, "vb": vb_dev}
        )

    res = run_bass_kernel_spmd(
        nc, in_maps, core_ids=list(range(N_CORES)), trace=_trace
    )
    if _trace:
        _NC_CACHE["last_result"] = res

    att = np.concatenate(
        [
            res.results[c]["att"].transpose(0, 2, 1).reshape(B_LOC, SRC_LEN)
            for c in range(N_CORES)
        ],
        axis=0,
    ).astype(np.float32)  # [B, S] raw fp8-path logits

    # host correction: subtract the exactly-computable coherent part of
    # the fp8 quantization error, weighted by E[tanh' | b, h]
    w8_deq = w8_q.astype(np.float32) / np.float32(W_SCALE)     # [H, E]
    sig_h = np.linalg.norm(w_e, axis=1)                        # [H]
    xs, ws_gh = np.polynomial.hermite_e.hermegauss(21)
    z = sig_h[None, :, None] * xs[None, None, :] + c_b[:, :, None]
    c_bh = (np.cosh(z) ** -2 * ws_gh[None, None, :]).sum(-1) / np.sqrt(
        2 * np.pi
    )                                                          # [B, H]
    for b in range(BATCH):
        vt = (v_w * c_bh[b]).astype(np.float64)
        g8 = w8_deq.T.astype(np.float64) @ vt                  # [E]
        g0 = w_e.T.astype(np.float64) @ vt
        e8b = e8_q[:, b, :].astype(np.float64) / ENC_SCALE     # [S, E]
        encb = encoder_outputs[:, b, :].astype(np.float64)
        att[b] -= (e8b @ g8 - encb @ g0).astype(np.float32)

    m = att.max(axis=1, keepdims=True)
    e = np.exp(att - m)
    out = e / e.sum(axis=1, keepdims=True)
    return out.astype(np.float32)


# revision 15
# speedup vs baseline: 1.0049x; 1.0049x over previous
"""Trainium2 Bass kernel for nn_Attention (Bahdanau-style additive attention).

Reference computation:
    enc = encoder_outputs.transpose(1, 0, 2)            # [B, S, 2H]
    e_proj = enc @ w_e.T                                # [B, S, H]
    energy = tanh(h_proj[:, None, :] + e_proj + b)      # [B, S, H]
    att = energy @ v_w                                  # [B, S]
    out = softmax(att, axis=1)

Sharding: data-parallel over batch, 4 batch rows per core on 8 cores.

The dominant cost is the e_proj matmul (34.4 GFLOP/core). This version
runs it in fp8 (e4m3) with DoubleRow perf mode (2 fp8 MACs per PE cell
per cycle, K=256 per accumulation chunk), ~2x the bf16 matmul rate:
  - host pre-transposes + quantizes enc to fp8 [b, ec, p, i, s] tiles
    (e = ec*256 + i*128 + p), so the kernel does plain contiguous DMAs
  - w_e is quantized to fp8 [ec, p, i, h] and kept SBUF-resident
  - per s-tile: psum[s(128), h(512)] accumulates 8 DoubleRow matmuls
    (lhsT = enc chunk [128, 2, 128] stationary, rhs = w chunk
    [128, 2, 512] moving), two h-groups = two psum banks
  - epilogue off the PE: DVE adds the (scaled) bias c_b = h_proj +
    attn_b in fp32, ACT applies tanh with a 2^-13 descale folded into
    its input scale, and DVE does the v-weighted reduction (tensor_mul
    + tensor_reduce) straight into the logit column
  - fp8 values are scaled into range (enc x16, w x512, both exact
    powers of two); the product scale 2^13 is removed by the ACT scale

fp8 quantization alone would put the softmax rel-err at ~1.9e-2 --
too close to the 2e-2 gate. The host therefore subtracts the COHERENT
part of the logit error, which is exactly computable with matvecs:
  delta_att(b,s) ~= sum_h v_h * tanh'(u) * delta_u(b,s,h)
                 ~= sum_h (v_h * E[tanh'|b,h]) * delta_u
  with sum_h vt_h * delta_u = enc8_row . (w8^T vt) - enc_row . (w^T vt)
E[tanh'|b,h] is a 1D Gaussian integral (Gauss-Hermite) since
u(b,s,h) ~ N(c_b[b,h], ||w_e[h]||^2) over s. This cuts the measured
rel-err to ~5.6e-3 in simulation. h_proj and the final softmax are
tiny and run on the host in fp32.
"""

import sys

try:
    import concourse.bass as bass  # noqa: F401
except ImportError:
    sys.path.insert(0, "/opt/trn_rl_repo")

import numpy as np
import ml_dtypes

import concourse.bacc as bacc
import concourse.mybir as mybir
import concourse.tile as tile
from concourse.bass_utils import run_bass_kernel_spmd

HID = 1024
BATCH = 32
SRC_LEN = 2048

N_CORES = 8
B_LOC = BATCH // N_CORES      # 4
E = 2 * HID                   # 2048
N_EC = E // 256               # 8 e-chunks of 256 (DoubleRow K)
N_ST = SRC_LEN // 128         # 16 s-tiles per batch row
HG = 512                      # h per psum bank
N_HG = HID // HG              # 2 h-groups
N_RAMP = 4                    # s-tiles computed e-chunk-major at startup

ENC_SCALE = 16.0              # exact powers of two
W_SCALE = 512.0
INV_SC = 1.0 / (ENC_SCALE * W_SCALE)

f32 = mybir.dt.float32
bf16 = mybir.dt.bfloat16
f8 = mybir.dt.float8e4
DR = mybir.MatmulPerfMode.DoubleRow

_NC_CACHE = {}


def _build():
    nc = bacc.Bacc(
        "TRN2", target_bir_lowering=False, debug=False, num_devices=N_CORES
    )
    enc8 = nc.declare_dram_parameter(
        "enc8", [B_LOC, N_EC, 128, 2, SRC_LEN], f8, isOutput=False
    )
    w8 = nc.declare_dram_parameter("w8", [N_EC, 128, 2, HID], f8, isOutput=False)
    cbb = nc.declare_dram_parameter("cbb", [B_LOC, 128, HID], f32, isOutput=False)
    vb = nc.declare_dram_parameter("vb", [128, HID], bf16, isOutput=False)
    # [b, p, st]: logit(b, st*128 + p)
    att = nc.declare_dram_parameter("att", [B_LOC, 128, N_ST], f32, isOutput=True)

    with tile.TileContext(nc) as tc:
        with (
            tc.tile_pool(name="const", bufs=1) as const_pool,
            tc.tile_pool(name="cbbp", bufs=2) as cbb_pool,
            tc.tile_pool(name="encp", bufs=16) as enc_pool,
            tc.tile_pool(name="tanhE", bufs=4) as te_pool,
            tc.tile_pool(name="prep", bufs=4) as pre_pool,
            tc.tile_pool(name="scr", bufs=3) as sc_pool,
            tc.tile_pool(name="attsb", bufs=1) as att_pool,
            tc.tile_pool(name="psum", bufs=8, space="PSUM") as psum_pool,
        ):
            w_sb = const_pool.tile([128, N_EC, 2, HID], f8)
            vb_sb = const_pool.tile([128, HID], bf16)
            att_sb = att_pool.tile([128, B_LOC * N_ST], f32)

            enc_ts = {}
            cbb_sbs = [None] * B_LOC

            def load_w(ec, split=False):
                if split:
                    for hg in range(N_HG):
                        nc.sync.dma_start(
                            w_sb[:, ec, :, hg * HG:(hg + 1) * HG],
                            w8[ec, :, :, hg * HG:(hg + 1) * HG],
                        )
                else:
                    nc.sync.dma_start(w_sb[:, ec], w8[ec])

            def load_enc(b, ec):
                t = enc_pool.tile(
                    [128, 2, SRC_LEN], f8, tag="enc", name=f"enc_{b}_{ec}"
                )
                nc.sync.dma_start(t[:], enc8[b, ec])
                enc_ts[(b, ec)] = t

            QS = SRC_LEN // 4     # ramp DMA quarter (= s-tiles 4q..4q+3)

            def load_enc_q(b, ec, q):
                if q == 0:
                    t = enc_pool.tile(
                        [128, 2, SRC_LEN], f8, tag="enc", name=f"enc_{b}_{ec}"
                    )
                    enc_ts[(b, ec)] = t
                t = enc_ts[(b, ec)]
                nc.sync.dma_start(
                    t[:, :, q * QS:(q + 1) * QS],
                    enc8[b, ec, :, :, q * QS:(q + 1) * QS],
                )

            def load_cbb(b):
                t = cbb_pool.tile([128, HID], f32, tag="cbb", name=f"cbb_{b}")
                nc.sync.dma_start(t[:], cbb[b])
                cbb_sbs[b] = t

            # startup DMAs in small e-chunk waves (w[ec] split by h-group +
            # the first enc s-quarter, ~384KB/wave ~= the PE's per-chunk
            # ramp consumption) so the first matmul starts after one wave
            # and never starves through the ramp
            for ec in range(N_EC):
                load_w(ec, split=True)
                load_enc_q(0, ec, 0)
            load_cbb(0)
            nc.sync.dma_start(vb_sb[:], vb[:])
            for q in range(1, 4):
                for ec in range(N_EC):
                    load_enc_q(0, ec, q)

            # warmup tanh for the ACT LUT-table dependency
            warm = const_pool.tile([128, 1], f32)
            nc.scalar.activation(
                warm[:], vb_sb[:, 0:1], mybir.ActivationFunctionType.Tanh
            )

            def mm(ps_hg, b, st, ec):
                lhsT = enc_ts[(b, ec)][:, :, st * 128:(st + 1) * 128]
                for hg in range(N_HG):
                    nc.tensor.matmul(
                        ps_hg[hg][:],
                        lhsT=lhsT,
                        rhs=w_sb[:, ec, :, hg * HG:(hg + 1) * HG],
                        start=(ec == 0),
                        stop=(ec == N_EC - 1),
                        perf_mode=DR,
                    )

            def epilogue(b, st, ps_hg):
                te = te_pool.tile([128, HID], bf16, tag="te", name=f"te_{b}_{st}")
                for hg in range(N_HG):
                    pre = pre_pool.tile(
                        [128, HG], f32, tag="pre", name=f"pre_{b}_{st}_{hg}"
                    )
                    nc.vector.tensor_add(
                        out=pre[:],
                        in0=ps_hg[hg][:],
                        in1=cbb_sbs[b][:, hg * HG:(hg + 1) * HG],
                    )
                    nc.scalar.activation(
                        te[:, hg * HG:(hg + 1) * HG], pre[:],
                        mybir.ActivationFunctionType.Tanh,
                        scale=INV_SC,
                    )
                col = b * N_ST + st
                prod = sc_pool.tile(
                    [128, HID], bf16, tag="ttr", name=f"ttr_{b}_{st}"
                )
                nc.vector.tensor_mul(out=prod[:], in0=te[:], in1=vb_sb[:])
                nc.vector.tensor_reduce(
                    att_sb[:, col:col + 1],
                    prod[:],
                    mybir.AxisListType.X,
                    mybir.AluOpType.add,
                )

            def psum_pair(b, st):
                return [
                    psum_pool.tile([128, HG], f32, tag="ps", name=f"ps_{b}_{st}_{g}")
                    for g in range(N_HG)
                ]

            # ---- batch row 0 ramp: first N_RAMP s-tiles e-chunk-major so
            # the PE starts as soon as (w8[0], enc[0,0]) land instead of
            # waiting for the whole row
            ramp_ps = [psum_pair(0, st) for st in range(N_RAMP)]
            for ec in range(N_EC):
                for st in range(N_RAMP):
                    mm(ramp_ps[st], 0, st, ec)
            for st in range(N_RAMP):
                epilogue(0, st, ramp_ps[st])

            # ---- steady state ----
            for b in range(B_LOC):
                for st in range(0 if b else N_RAMP, N_ST):
                    # prefetch next row's tiles mid-row
                    if b < B_LOC - 1:
                        if st == 4:
                            load_cbb(b + 1)
                        if 4 <= st < 4 + N_EC:
                            load_enc(b + 1, st - 4)
                    ps_hg = psum_pair(b, st)
                    for ec in range(N_EC):
                        mm(ps_hg, b, st, ec)
                    epilogue(b, st, ps_hg)
                nc.sync.dma_start(att[b], att_sb[:, b * N_ST:(b + 1) * N_ST])
    nc.compile()
    return nc


def _get_nc():
    if "nc" not in _NC_CACHE:
        _NC_CACHE["nc"] = _build()
    return _NC_CACHE["nc"]


def kernel(hidden, encoder_outputs, attn_w, attn_b, v_w, _trace=False):
    hidden = np.asarray(hidden, dtype=np.float32)
    encoder_outputs = np.asarray(encoder_outputs, dtype=np.float32)
    attn_w = np.asarray(attn_w, dtype=np.float32)
    attn_b = np.asarray(attn_b, dtype=np.float32)
    v_w = np.asarray(v_w, dtype=np.float32)

    c_b = hidden @ attn_w[:, :HID].T + attn_b          # [B, H] fp32
    w_e = np.ascontiguousarray(attn_w[:, HID:])        # [H, E]

    # fp8 quantization (scales are exact powers of two)
    w8_q = (w_e * np.float32(W_SCALE)).astype(ml_dtypes.float8_e4m3)   # [H, E]
    e8_q = (encoder_outputs * np.float32(ENC_SCALE)).astype(
        ml_dtypes.float8_e4m3
    )                                                                   # [S, B, E]

    # device weight layout [ec, p, i, h], e = ec*256 + i*128 + p
    w8_dev = np.ascontiguousarray(
        w8_q.T.reshape(N_EC, 2, 128, HID).transpose(0, 2, 1, 3)
    )
    vb_dev = np.ascontiguousarray(
        np.broadcast_to(v_w[None, :], (128, HID))
    ).astype(ml_dtypes.bfloat16)

    nc = _get_nc()
    sc = np.float32(ENC_SCALE * W_SCALE)
    in_maps = []
    for core in range(N_CORES):
        b0 = core * B_LOC
        enc_dev = np.ascontiguousarray(
            e8_q[:, b0:b0 + B_LOC, :].transpose(1, 2, 0)
            .reshape(B_LOC, N_EC, 2, 128, SRC_LEN).transpose(0, 1, 3, 2, 4)
        )
        cbb_dev = np.ascontiguousarray(
            np.broadcast_to(
                (c_b[b0:b0 + B_LOC] * sc)[:, None, :], (B_LOC, 128, HID)
            )
        ).astype(np.float32)
        in_maps.append(
            {"enc8": enc_dev, "w8": w8_dev, "cbb": cbb_dev, "vb": vb_dev}
        )

    res = run_bass_kernel_spmd(
        nc, in_maps, core_ids=list(range(N_CORES)), trace=_trace
    )
    if _trace:
        _NC_CACHE["last_result"] = res

    att = np.concatenate(
        [
            res.results[c]["att"].transpose(0, 2, 1).reshape(B_LOC, SRC_LEN)
            for c in range(N_CORES)
        ],
        axis=0,
    ).astype(np.float32)  # [B, S] raw fp8-path logits

    # host correction: subtract the exactly-computable coherent part of
    # the fp8 quantization error, weighted by E[tanh' | b, h]
    w8_deq = w8_q.astype(np.float32) / np.float32(W_SCALE)     # [H, E]
    sig_h = np.linalg.norm(w_e, axis=1)                        # [H]
    xs, ws_gh = np.polynomial.hermite_e.hermegauss(21)
    z = sig_h[None, :, None] * xs[None, None, :] + c_b[:, :, None]
    c_bh = (np.cosh(z) ** -2 * ws_gh[None, None, :]).sum(-1) / np.sqrt(
        2 * np.pi
    )                                                          # [B, H]
    for b in range(BATCH):
        vt = (v_w * c_bh[b]).astype(np.float64)
        g8 = w8_deq.T.astype(np.float64) @ vt                  # [E]
        g0 = w_e.T.astype(np.float64) @ vt
        e8b = e8_q[:, b, :].astype(np.float64) / ENC_SCALE     # [S, E]
        encb = encoder_outputs[:, b, :].astype(np.float64)
        att[b] -= (e8b @ g8 - encb @ g0).astype(np.float32)

    m = att.max(axis=1, keepdims=True)
    e = np.exp(att - m)
    out = e / e.sum(axis=1, keepdims=True)
    return out.astype(np.float32)
